# revision 10
# baseline (speedup 1.0000x reference)
"""Trainium2 Bass kernel for nn_LSTMActionAgent (B=512, S=256, I=12, H=256, A=9).

Sharding: data-parallel over batch across 8 cores (64 rows each); parameters
replicated. Everything on-chip runs in "transposed" layout: feature dims on
SBUF partitions, batch on the free axis, so the LSTM recurrence needs no
per-step transposes.

Per core:
  phase 1  embed: emb^T = relu(W_emb @ x^T)          (bf16, full seq in SBUF)
  phase 2  two skewed LSTM scans, chunked (C=2 steps):
             - input-gate batch matmuls write PSUM (start=True) a chunk ahead
             - per-step recurrent matmuls accumulate on top (start=False)
             - sigmoid/tanh read PSUM, cell update on DVE/GpSimd fp32
  phase 3  head: LayerNorm (PE column-sum + PE broadcast), MLP, mask.
"""

import numpy as np
import ml_dtypes

import concourse.bass as bass
import concourse.mybir as mybir
import concourse.tile as tile
from concourse import bacc
from concourse import bass_utils
from concourse.bass import ts, ds

F32 = mybir.dt.float32
BF16 = mybir.dt.bfloat16
AF = mybir.ActivationFunctionType
OP = mybir.AluOpType

B, S, I, H, A = 512, 256, 12, 256, 9
NCORES = 8
BC = B // NCORES          # 64 batch rows per core
KT = H // 128             # 2 k-tiles of hidden dim
MT = (4 * H) // 128       # 8 m-tiles of gate dim
C = 2                     # scan chunk length (steps)
NCH = S // C              # 128 chunks
SB = S * BC               # 16384 flattened (t, b) columns, t-major

# gate-dim row permutation: PyTorch order [i f g o] -> mtile order [i f o g]
# so sigmoid gates (i, f, o) are contiguous (mt 0..5) and tanh gate g is mt 6..7
GATE_PERM = np.concatenate([np.arange(0, 512), np.arange(768, 1024), np.arange(512, 768)])


def _bf(a):
    return np.ascontiguousarray(a.astype(ml_dtypes.bfloat16))


def _f32(a):
    return np.ascontiguousarray(a.astype(np.float32))


def _wT_tiles(w):
    """[4H, H] weight -> lhsT tile layout [128, KT, MT, 128] flat [128, KT*MT*128].

    lhsT[kt][p, m] view such that (lhsT.T @ hT)[m, n] = sum_k w_perm[mt*128+m, kt*128+k] h[k, n].
    """
    wp = w[GATE_PERM]                       # [1024, 256]
    wt = wp.T                               # [256, 1024] = [kt*128+p, mt*128+m]
    wt = wt.reshape(KT, 128, MT, 128)       # [kt, p, mt, m]
    wt = wt.transpose(1, 0, 2, 3)           # [p, kt, mt, m]
    return _bf(wt.reshape(128, KT * MT * 128))


def _build(num_devices):
    nc = bacc.Bacc("TRN2", target_bir_lowering=False, debug=False,
                   num_devices=num_devices)

    # ---- DRAM I/O ----------------------------------------------------------
    d_xT = nc.dram_tensor("xT", [I, SB], BF16, kind="ExternalInput").ap()
    d_wemb = nc.dram_tensor("wembT", [I, H], BF16, kind="ExternalInput").ap()
    d_bemb = nc.dram_tensor("bembC", [128, KT], F32, kind="ExternalInput").ap()
    d_wih = [nc.dram_tensor(f"wih{l}", [128, KT * MT * 128], BF16, kind="ExternalInput").ap()
             for l in range(2)]
    d_whh = [nc.dram_tensor(f"whh{l}", [128, KT * MT * 128], BF16, kind="ExternalInput").ap()
             for l in range(2)]
    d_lng = nc.dram_tensor("lngC", [128, KT], F32, kind="ExternalInput").ap()
    d_lnb = nc.dram_tensor("lnbC", [128, KT], F32, kind="ExternalInput").ap()
    d_w1 = nc.dram_tensor("w1T", [128, KT * 32], BF16, kind="ExternalInput").ap()
    d_b1 = nc.dram_tensor("b1C", [32, 1], F32, kind="ExternalInput").ap()
    d_w2 = nc.dram_tensor("w2T", [32, A], BF16, kind="ExternalInput").ap()
    d_b2m = nc.dram_tensor("b2mC", [A, 1], F32, kind="ExternalInput").ap()
    d_mask = nc.dram_tensor("maskT", [A, BC], F32, kind="ExternalInput").ap()

    d_out_hc = nc.dram_tensor("out_hc", [4, H, BC], F32, kind="ExternalOutput").ap()
    d_out_log = nc.dram_tensor("out_logits", [A, BC], F32, kind="ExternalOutput").ap()

    with tile.TileContext(nc) as tc:
        _emit(tc, nc, d_xT, d_wemb, d_bemb, d_wih, d_whh, d_lng, d_lnb,
              d_w1, d_b1, d_w2, d_b2m, d_mask, d_out_hc, d_out_log)

    nc.compile()
    return nc


def _emit(tc, nc, d_xT, d_wemb, d_bemb, d_wih, d_whh, d_lng, d_lnb,
          d_w1, d_b1, d_w2, d_b2m, d_mask, d_out_hc, d_out_log):
    from contextlib import ExitStack

    ctx = ExitStack()
    with ctx:
        consts = ctx.enter_context(tc.tile_pool(name="consts", bufs=1))
        big = ctx.enter_context(tc.tile_pool(name="big", bufs=1))

        # ---- constants into SBUF ------------------------------------------
        wemb_sb = consts.tile([I, H], BF16)
        nc.sync.dma_start(out=wemb_sb, in_=d_wemb)
        bemb_sb = consts.tile([128, KT], F32)
        nc.sync.dma_start(out=bemb_sb, in_=d_bemb)
        wih_sb = []
        whh_sb = []
        for l in range(2):
            t = consts.tile([128, KT, MT, 128], BF16, tag=f"wih{l}", name=f"wihsb{l}")
            nc.sync.dma_start(out=t, in_=d_wih[l].rearrange("p (kt mt m) -> p kt mt m", kt=KT, mt=MT))
            wih_sb.append(t)
            t = consts.tile([128, KT, MT, 128], BF16, tag=f"whh{l}", name=f"whhsb{l}")
            nc.sync.dma_start(out=t, in_=d_whh[l].rearrange("p (kt mt m) -> p kt mt m", kt=KT, mt=MT))
            whh_sb.append(t)
        lng_sb = consts.tile([128, KT], F32)
        nc.sync.dma_start(out=lng_sb, in_=d_lng)
        lnb_sb = consts.tile([128, KT], F32)
        nc.sync.dma_start(out=lnb_sb, in_=d_lnb)
        w1_sb = consts.tile([128, KT, 32], BF16)
        nc.sync.dma_start(out=w1_sb, in_=d_w1.rearrange("p (kt m) -> p kt m", kt=KT))
        b1_sb = consts.tile([32, 1], F32)
        nc.sync.dma_start(out=b1_sb, in_=d_b1)
        w2_sb = consts.tile([32, A], BF16)
        nc.sync.dma_start(out=w2_sb, in_=d_w2)
        b2m_sb = consts.tile([A, 1], F32)
        nc.sync.dma_start(out=b2m_sb, in_=d_b2m)
        mask_sb = consts.tile([A, BC], F32)
        nc.sync.dma_start(out=mask_sb, in_=d_mask)

        hz_sb = consts.tile([128, KT, BC], BF16)   # zero h_{-1}
        nc.vector.memset(hz_sb, 0.0)
        onesK = consts.tile([128, 1], BF16)        # column of 1/H for mean
        nc.vector.memset(onesK, 1.0 / H)
        ones1 = consts.tile([1, 128], BF16)        # row of ones for broadcast
        nc.vector.memset(ones1, 1.0)
        eps_sb = consts.tile([1, 1], F32)
        nc.vector.memset(eps_sb, 0.0)              # bias slot for sqrt(var+eps)
        nc.vector.memset(eps_sb, 1e-5)

        # ---- big persistent SBUF ------------------------------------------
        embT = big.tile([128, KT, SB], BF16)       # relu(Wemb x)^T, full seq
        y0T = big.tile([128, KT, SB], BF16)        # layer-0 outputs, full seq
        c_sb = [big.tile([128, KT, BC], F32, tag=f"c{l}", name=f"c{l}") for l in range(2)]
        for l in range(2):
            nc.vector.memset(c_sb[l], 0.0)
        h1ring = big.tile([128, KT, 4, BC], BF16)  # layer-1 h ring
        hout_f = [big.tile([128, KT, BC], F32, tag=f"hout{l}", name=f"hout{l}") for l in range(2)]

        # ---- phase 1: embed ----------------------------------------------
        EC = 512  # embed column chunk
        with tc.tile_pool(name="embp", bufs=2, space="PSUM") as embp, \
             tc.tile_pool(name="xtp", bufs=3) as xtp:
            for cc in range(SB // EC):
                xt = xtp.tile([I, EC], BF16, tag="xt")
                nc.sync.dma_start(out=xt, in_=d_xT[:, ts(cc, EC)])
                for kt in range(KT):
                    ps = embp.tile([128, EC], F32, tag="eps")
                    nc.tensor.matmul(ps, wemb_sb[:, ts(kt, 128)], xt,
                                     start=True, stop=True)
                    nc.scalar.activation(embT[:, kt, ts(cc, EC)], ps, AF.Relu,
                                         bias=bemb_sb[:, kt:kt + 1])

        # ---- phase 2: skewed scans ----------------------------------------
        # per layer, per chunk parity: PSUM tile [128, MT, C, BC] = 2 banks
        work = ctx.enter_context(tc.tile_pool(name="work", bufs=3))
        scanp_cm = tc.tile_pool(name="scanp", bufs=2, space="PSUM")
        scanp = scanp_cm.__enter__()

        gpsum = [[None, None], [None, None]]  # [layer][parity]

        def xg_chunk(l, k):
            """Batched input-gate matmuls for chunk k of layer l -> PSUM."""
            src = embT if l == 0 else y0T
            ps = scanp.tile([128, MT, C, BC], F32, tag=f"g{l}")
            gpsum[l][k % 2] = ps
            for mt in range(MT):
                for kt in range(KT):
                    # start=True clears has_written for the WHOLE bank, so only
                    # the first matmul touching each PSUM bank may set it.
                    nc.tensor.matmul(
                        ps[:, mt], wih_sb[l][:, kt, mt], src[:, kt, ts(k, C * BC)],
                        start=(kt == 0 and mt % 4 == 0), stop=False,
                        skip_group_check=True)

        def step(l, gt):
            """One LSTM step for layer l at global step gt."""
            ps = gpsum[l][(gt // C) % 2]
            t = gt % C
            if gt == 0:
                hprev = hz_sb
            elif l == 0:
                hprev = y0T[:, :, ts(gt - 1, BC)]
            else:
                hprev = h1ring[:, :, (gt - 1) % 4, :]
            for mt in range(MT):
                for kt in range(KT):
                    nc.tensor.matmul(
                        ps[:, mt, t], whh_sb[l][:, kt, mt], hprev[:, kt],
                        start=False, stop=(kt == KT - 1), skip_group_check=True)

            ifo = work.tile([128, 6, BC], F32, tag=f"ifo{l}")
            nc.scalar.activation(ifo, ps[:, 0:6, t], AF.Sigmoid)
            gg = work.tile([128, KT, BC], F32, tag=f"g{l}")
            nc.scalar.activation(gg, ps[:, 6:8, t], AF.Tanh)
            t2 = work.tile([128, KT, BC], F32, tag=f"t2{l}")
            nc.gpsimd.tensor_mul(t2, ifo[:, 0:2], gg)
            t1 = work.tile([128, KT, BC], F32, tag=f"t1{l}")
            nc.vector.tensor_mul(t1, ifo[:, 2:4], c_sb[l])
            nc.vector.tensor_add(c_sb[l], t1, t2)
            tc_ = work.tile([128, KT, BC], F32, tag=f"tc{l}")
            nc.scalar.activation(tc_, c_sb[l], AF.Tanh)
            if l == 0:
                hdst = y0T[:, :, ts(gt, BC)]
            else:
                hdst = h1ring[:, :, gt % 4, :]
            nc.vector.tensor_mul(hdst, ifo[:, 4:6], tc_)
            if gt == S - 1:
                nc.vector.tensor_mul(hout_f[l], ifo[:, 4:6], tc_)

        # skew: l0 runs one chunk ahead of l1
        for k in range(NCH + 1):
            if k < NCH:
                xg_chunk(0, k)
                for t in range(C):
                    step(0, k * C + t)
            if k >= 1:
                xg_chunk(1, k - 1)
                for t in range(C):
                    step(1, (k - 1) * C + t)

        scanp_cm.__exit__(None, None, None)

        # write h/c outputs: [h0, c0, h1, c1] as [H, BC] fp32
        for l in range(2):
            nc.sync.dma_start(
                out=d_out_hc[2 * l].rearrange("(kt p) b -> p kt b", p=128),
                in_=hout_f[l])
            nc.sync.dma_start(
                out=d_out_hc[2 * l + 1].rearrange("(kt p) b -> p kt b", p=128),
                in_=c_sb[l])

        # ---- phase 3: head -------------------------------------------------
        with tc.tile_pool(name="headp", bufs=1, space="PSUM") as headp, \
             tc.tile_pool(name="heads", bufs=1) as heads:
            h_last_bf = h1ring[:, :, (S - 1) % 4, :]
            # mean and mean-of-squares via PE column reduction
            mu_ps = headp.tile([1, BC], F32, tag="mu")
            sq_bf = heads.tile([128, KT, BC], BF16)
            nc.scalar.activation(sq_bf, hout_f[1], AF.Square)
            musq_ps = headp.tile([1, BC], F32, tag="musq")
            for kt in range(KT):
                nc.tensor.matmul(mu_ps, onesK, h_last_bf[:, kt],
                                 start=(kt == 0), stop=(kt == KT - 1))
                nc.tensor.matmul(musq_ps, onesK, sq_bf[:, kt],
                                 start=(kt == 0), stop=(kt == KT - 1))
            mu_sb = heads.tile([1, BC], F32)
            nc.vector.tensor_copy(mu_sb, mu_ps)
            musq_sb = heads.tile([1, BC], F32)
            nc.vector.tensor_copy(musq_sb, musq_ps)
            # var = E[x^2] - mu^2 ; rstd = 1/sqrt(var + eps)
            var_sb = heads.tile([1, BC], F32)
            nc.vector.scalar_tensor_tensor(var_sb, mu_sb, -1.0, mu_sb, OP.mult, OP.mult)
            nc.vector.tensor_add(var_sb, var_sb, musq_sb)
            sd_sb = heads.tile([1, BC], F32)
            nc.scalar.activation(sd_sb, var_sb, AF.Sqrt, bias=eps_sb)
            rstd_sb = heads.tile([1, BC], F32)
            nc.vector.reciprocal(rstd_sb, sd_sb)
            # pack [rstd | -mu*rstd] and broadcast to 128 partitions via PE
            bc2 = heads.tile([1, 2, BC], BF16)
            nc.vector.tensor_copy(bc2[:, 0], rstd_sb)
            nc.vector.scalar_tensor_tensor(bc2[:, 1], mu_sb, -1.0, rstd_sb,
                                           OP.mult, OP.mult)
            bc_ps = headp.tile([128, 2 * BC], F32, tag="bc")
            nc.tensor.matmul(bc_ps, ones1, bc2.rearrange("o a b -> o (a b)"),
                             start=True, stop=True)
            bc_sb = heads.tile([128, 2, BC], F32)
            nc.vector.tensor_copy(bc_sb, bc_ps)
            # normed = ((h - mu) * rstd) * ln_g + ln_b
            normed = heads.tile([128, KT, BC], BF16)
            for kt in range(KT):
                n1 = heads.tile([128, BC], F32, tag=f"n1{kt}")
                nc.vector.tensor_mul(n1, hout_f[1][:, kt], bc_sb[:, 0])
                nc.vector.tensor_add(n1, n1, bc_sb[:, 1])
                nc.vector.tensor_scalar(normed[:, kt], n1,
                                        lng_sb[:, kt:kt + 1], lnb_sb[:, kt:kt + 1],
                                        OP.mult, OP.add)
            # MLP
            hdn_ps = headp.tile([32, BC], F32, tag="hdn")
            for kt in range(KT):
                nc.tensor.matmul(hdn_ps, w1_sb[:, kt], normed[:, kt],
                                 start=(kt == 0), stop=(kt == KT - 1))
            hdn_bf = heads.tile([32, BC], BF16)
            nc.scalar.activation(hdn_bf, hdn_ps, AF.Relu, bias=b1_sb)
            log_ps = headp.tile([A, BC], F32, tag="log")
            nc.tensor.matmul(log_ps, w2_sb, hdn_bf, start=True, stop=True)
            log_sb = heads.tile([A, BC], F32)
            nc.vector.tensor_scalar(log_sb, log_ps, b2m_sb, None, OP.add)
            # logits + (1-mask)*(-1e9), computing (1-mask) first so the
            # penalty is exactly 0 where mask==1 (no 1e9 cancellation).
            mask1m = heads.tile([A, BC], F32)
            nc.vector.tensor_scalar(mask1m, mask_sb, -1.0, 1.0, OP.mult, OP.add)
            out_log = heads.tile([A, BC], F32)
            nc.vector.scalar_tensor_tensor(out_log, mask1m, -1e9, log_sb,
                                           OP.mult, OP.add)
            nc.sync.dma_start(out=d_out_log, in_=out_log)


# --------------------------------------------------------------------------
_CACHE = {}


def _get_program(num_devices):
    if num_devices not in _CACHE:
        _CACHE[num_devices] = _build(num_devices)
    return _CACHE[num_devices]


def prep_in_maps(inputs):
    """Host-side layout prep; returns per-core in_maps for the bass program."""
    x = _f32(np.asarray(inputs["x"]))
    mask = _f32(np.asarray(inputs["mask"]))
    wemb = _f32(np.asarray(inputs["W_emb"]))
    bemb = _f32(np.asarray(inputs["b_emb"]))
    lng = _f32(np.asarray(inputs["ln_g"]))
    lnb = _f32(np.asarray(inputs["ln_b"]))
    w1 = _f32(np.asarray(inputs["w1"]))
    b1 = _f32(np.asarray(inputs["b1"]))
    w2 = _f32(np.asarray(inputs["w2"]))
    b2 = _f32(np.asarray(inputs["b2"]))

    shared = {
        "wembT": _bf(wemb.T),                                   # [12, 256]
        "bembC": _f32(bemb.reshape(KT, 128).T),                 # [128, 2]
        "lngC": _f32(lng.reshape(KT, 128).T),
        "lnbC": _f32(lnb.reshape(KT, 128).T),
        "w1T": _bf(w1.T.reshape(KT, 128, 32).transpose(1, 0, 2).reshape(128, KT * 32)),
        "b1C": _f32(b1.reshape(32, 1)),
        "w2T": _bf(w2.T),                                       # [32, 9]
        "b2mC": _f32(b2.reshape(A, 1)),
    }
    for l in range(2):
        w_ih = _f32(np.asarray(inputs[f"w_ih{l}"]))
        w_hh = _f32(np.asarray(inputs[f"w_hh{l}"]))
        b_ih = np.asarray(inputs[f"b_ih{l}"])
        b_hh = np.asarray(inputs[f"b_hh{l}"])
        assert np.all(b_ih == 0) and np.all(b_hh == 0), \
            "nonzero LSTM biases not supported by this kernel build"
        shared[f"wih{l}"] = _wT_tiles(w_ih)
        shared[f"whh{l}"] = _wT_tiles(w_hh)

    in_maps = []
    for c in range(NCORES):
        xc = x[c * BC:(c + 1) * BC]                 # [64, 256, 12]
        xT = xc.transpose(2, 1, 0).reshape(I, SB)   # [12, (t-major) 16384]
        m = dict(shared)
        m["xT"] = _bf(xT)
        m["maskT"] = _f32(mask[c * BC:(c + 1) * BC].T)
        in_maps.append(m)
    return in_maps


def run_raw(inputs, trace=False, **kw):
    nc = _get_program(NCORES)
    in_maps = prep_in_maps(inputs)
    res = bass_utils.run_bass_kernel_spmd(
        nc, in_maps, core_ids=list(range(NCORES)), trace=trace, **kw)
    return res


def assemble(results):
    logits = np.zeros((B, A), np.float32)
    h = np.zeros((2, B, H), np.float32)
    c = np.zeros((2, B, H), np.float32)
    for ci, r in enumerate(results):
        sl = slice(ci * BC, (ci + 1) * BC)
        logits[sl] = r["out_logits"].T
        hc = r["out_hc"]                   # [4, H, BC]
        h[0, sl] = hc[0].T
        c[0, sl] = hc[1].T
        h[1, sl] = hc[2].T
        c[1, sl] = hc[3].T
    return logits, (h, c)


def kernel(**inputs):
    res = run_raw(inputs, trace=False)
    return assemble(res.results)
